# revision 35
# baseline (speedup 1.0000x reference)
"""Trainium2 Bass kernel for BackprojectDepth.

out[b, i, y*W+x] = depth[b, 0, y, x] * (K[b,i,0]*(x+dx[b]) + K[b,i,1]*(y+dy[b]) + K[b,i,2])   for i in 0..2
out[b, 3, :]    = 1.0

Sharding: pure data parallel over batch (32 batches -> 4 per core on 8 cores).

Memory-bound; the device program minimizes wire bytes (fp16 depth in, fp16
planes 0..2 out, constant ones-plane filled host-side during the gather =>
16 MB/core instead of 40 MB) and keeps the ~360 GB/s wire busy end-to-end:

  * layout: partition p holds image rows 4p..4p+3 (depth[b] / out[b,i] are
    the plain row-major reshape [128, 4096]); outs move as half-plane
    [128, 2048] DMAs (4 KB per-partition descriptors) so the out stream
    starts early and flows smoothly.
  * planes 0/1: lin = xg*A + bias on the scalar (ACT) engine per 1024-col
    chunk (int32 x-ramp input - measured faster than fp16 - fp16 out,
    1135 ns/op), then half-plane DVE multiplies by depth (~1.4 us each).
  * plane 2: lin on the tensor engine as K=2 matmuls (stationary [1; p]
    fixed, moving [A*x + B*q + c'; 4B] host-precomputed per (b,q)) into
    4-bank PSUM tiles, drained by [128,2048] DVE multiplies (~2.2 us).

Measured engine budgets/core: ACT 36.3 us, DVE ~39 us, PE ~22 us, wire
~45 us => DMA-bound when the pipeline stays dense.

Rings: sync = xg + depth[0] + plane 0/1 outs; scalar = consts + depth
prefetches (issued up-front, before any out can block them); gpsimd SWDGE
= plane 2 outs (gpsimd does no other work, avoiding SBUF contention).
"""

import numpy as np

import concourse.tile as tile
from concourse import bacc, mybir
from concourse.bass_utils import run_bass_kernel_spmd

N_CORES = 8
B, H, W = 32, 512, 1024
HW = H * W
BPC = B // N_CORES          # batches per core
RPP = H // 128              # image rows per partition (4)
CH = RPP * W                # cols per partition in plane layout (4096)
HC = CH // 2                # half-plane cols (2048)

F32 = mybir.dt.float32
F16 = mybir.dt.float16
I32 = mybir.dt.int32
I8 = mybir.dt.int8

_TRACE = False              # test.py may flip this for profiling
_LAST_RESULTS = None        # BassKernelResults from the last run (for test.py)

_nc_cache = None

DEFAULT_CFG = dict(
    dpool=4, opool=4, lpool=4, psum=2,
    plane_ring=("sync", "sync", "gpsimd"),
    o1h1_ring=None,          # optional override ring for plane-1 second half
)


def _build(**cfg_over):
    """Build + compile the per-core Bass program (SPMD: same NEFF, 8 cores)."""
    cfg = dict(DEFAULT_CFG, **cfg_over)
    nc = bacc.Bacc(
        "TRN2",
        target_bir_lowering=False,
        debug=False,
        enable_asserts=False,
        num_devices=N_CORES,
    )

    depth_d = nc.dram_tensor("depth", [BPC, H, W], F16, kind="ExternalInput")
    scale_d = nc.dram_tensor("scale", [128, BPC * 2], F32, kind="ExternalInput")
    bias_d = nc.dram_tensor("bias", [128, BPC * 2 * RPP], F32, kind="ExternalInput")
    stat_d = nc.dram_tensor("stat", [2, BPC * 128], F16, kind="ExternalInput")
    mov_d = nc.dram_tensor("mov", [2, BPC * RPP * W], F16, kind="ExternalInput")
    out_d = nc.dram_tensor("out", [BPC, 3, HW], I8, kind="ExternalOutput")

    rings = {"sync": nc.sync, "scalar": nc.scalar, "gpsimd": nc.gpsimd}

    with tile.TileContext(nc) as tc:
        with (
            tc.tile_pool(name="const", bufs=1) as cpool,
            tc.tile_pool(name="dpool", bufs=cfg["dpool"]) as dpool,
            tc.tile_pool(name="lpool", bufs=cfg["lpool"]) as lpool,
            tc.tile_pool(name="opool", bufs=cfg["opool"]) as opool,
            tc.psum_pool(name="ppool", bufs=cfg["psum"]) as ppool,
        ):
            # PE consts ride the gpsimd SWDGE ring: warms up its queue at
            # t~0 (else the first plane-2 out pays the ~7us cold start) and
            # keeps the scalar ring free for pure depth prefetch
            stat_t = cpool.tile([2, BPC * 128], F16)
            nc.gpsimd.dma_start(stat_t[:], stat_d.ap())
            mov_t = cpool.tile([2, BPC * RPP * W], F16)
            nc.gpsimd.dma_start(mov_t[:], mov_d.ap())
            # x-ramp on gpsimd (no DMA dependency); ACT converts i32 on read
            xg_t = cpool.tile([128, W], I32)
            nc.gpsimd.iota(xg_t[:], pattern=[[1, W]], base=0, channel_multiplier=0)
            sc_t = cpool.tile([128, BPC * 2], F32)
            nc.sync.dma_start(sc_t[:], scale_d.ap())
            bi_t = cpool.tile([128, BPC * 2 * RPP], F32)
            nc.sync.dma_start(bi_t[:], bias_d.ap())

            # partition p <-> image rows 4p..4p+3 (plain row-major reshape)
            depth_ap = depth_d.ap().rearrange("b (p q) w -> b p (q w)", p=128)
            out_ap = out_d.ap().rearrange("b i (p j) -> b i p j", p=128)

            # all depth loads issued up-front so prefetch never queues
            # behind an out-DMA on the same ring; depth[0] lands in column
            # quarters split over both HWDGE rings so the first chunk
            # arrives ~2us sooner
            d_ts = []
            for b in range(BPC):
                d_t = dpool.tile([128, CH], F16)
                if b == 0:
                    for qtr, deng in enumerate((nc.sync, nc.scalar, nc.sync, nc.scalar)):
                        sl = slice(qtr * W, (qtr + 1) * W)
                        deng.dma_start(d_t[:, sl], depth_ap[b, :, sl])
                else:
                    nc.scalar.dma_start(d_t[:], depth_ap[b])
                d_ts.append(d_t)

            def act_lin(b, i):
                l_t = lpool.tile([128, CH], F16)
                col = 2 * b + i
                for q in range(RPP):
                    nc.scalar.activation(
                        l_t[:, q * W : (q + 1) * W],
                        xg_t[:],
                        mybir.ActivationFunctionType.Identity,
                        bias=bi_t[:, col * RPP + q : col * RPP + q + 1],
                        scale=sc_t[:, col : col + 1],
                    )
                return l_t

            def mul_and_store(b, i, h, o_t, lin_ap, d_t):
                sl = slice(h * HC, (h + 1) * HC)
                nc.vector.tensor_mul(o_t[:, sl], lin_ap[:, sl], d_t[:, sl])
                ring = rings[cfg["plane_ring"][i]]
                if i == 1 and h == 1 and cfg["o1h1_ring"]:
                    ring = rings[cfg["o1h1_ring"]]
                ring.dma_start(out_ap[b, i, :, sl], o_t[:, sl])

            def pe_plane(b, d_t):
                o2 = opool.tile([128, CH], I8)
                for hf in range(2):
                    ps = ppool.tile([128, HC], F32)
                    for s in range(4):
                        c0 = hf * HC + s * 512
                        q, xo = c0 // W, c0 % W
                        nc.tensor.matmul(
                            ps[:, s * 512 : (s + 1) * 512],
                            stat_t[:, b * 128 : (b + 1) * 128],
                            mov_t[:, (b * RPP + q) * W + xo : (b * RPP + q) * W + xo + 512],
                            start=True,
                            stop=True,
                        )
                    sl = slice(hf * HC, (hf + 1) * HC)
                    nc.vector.tensor_mul(o2[:, sl], ps[:], d_t[:, sl])
                    rings[cfg["plane_ring"][2]].dma_start(out_ap[b, 2, :, sl], o2[:, sl])

            def act_plane(b, i, d_t, quarters=False):
                l_t = act_lin(b, i)
                o_t = opool.tile([128, CH], I8)
                if quarters:
                    # batch 0 plane 0: quarter-granularity so the first out
                    # bytes hit the wire as soon as the first depth quarter
                    # and lin chunk exist
                    for qtr in range(RPP):
                        sl = slice(qtr * W, (qtr + 1) * W)
                        nc.vector.tensor_mul(o_t[:, sl], l_t[:, sl], d_t[:, sl])
                        rings[cfg["plane_ring"][i]].dma_start(
                            out_ap[b, i, :, sl], o_t[:, sl]
                        )
                    return
                for h in range(2):
                    mul_and_store(b, i, h, o_t, l_t[:], d_t)

            for b in range(BPC):
                d_t = d_ts[b]
                act_plane(b, 0, d_t, quarters=(b == 0))
                pe_plane(b, d_t)
                act_plane(b, 1, d_t)

    nc.compile()
    return nc


def _make_in_maps(depth, inv_K, dxy):
    depth16 = np.ascontiguousarray(
        np.asarray(depth, dtype=np.float32).astype(np.float16)
    )
    K = np.asarray(inv_K, dtype=np.float64)
    dx = np.asarray(dxy, dtype=np.float64)

    # Per-batch affine coefficients: cam_i = A*x' + B*y' + C with x'=x+dx, y'=y+dy
    A = K[:, :3, 0]                                   # [B, 3]
    Bc = K[:, :3, 1]
    C = K[:, :3, 2]
    const = A * dx[:, None, 0] + Bc * dx[:, None, 1] + C   # [B, 3]

    p = np.arange(128, dtype=np.float64)
    q = np.arange(RPP, dtype=np.float64)
    x = np.arange(W, dtype=np.float64)

    # int8 output quantization: per (b, i, 4-row-group p) bound
    # s[b,i,p] = max |lin| over the group (lin affine in x,y => corners),
    # fold inv = 126/s into the lin tables so the device's existing
    # multiply emits pre-scaled int8; host dequantizes by s/126.
    xv = np.array([0.0, W - 1.0])
    yc = 4.0 * p[:, None] + np.array([0.0, 3.0])[None, :]          # [128, 2]
    lin_c = (
        A[:, :, None, None, None] * xv[None, None, None, None, :]
        + Bc[:, :, None, None, None] * yc[None, None, :, :, None]
        + const[:, :, None, None, None]
    )                                                  # [B, 3, 128, 2, 2]
    s_all = np.abs(lin_c).max(axis=(3, 4))             # [B, 3, 128]
    inv = np.minimum(126.0 / np.maximum(s_all, 1e-9), 500.0)
    scl = (1.0 / inv).astype(np.float32)               # host dequant factors

    # ACT path (planes 0/1): lin' = (A*inv_p)*x + (B*(4p+q)+const)*inv_p
    bias_all = (
        Bc[:, :2, None, None] * (4.0 * p[None, None, None, :] + q[None, None, :, None])
        + const[:, :2, None, None]
    ) * inv[:, :2, None, :]                            # [B, 2, RPP, 128]
    scale_all = (
        A[:, :2, None] * inv[:, :2, :]
    )                                                  # [B, 2, 128]
    # PE path (plane 2): stationary rows [inv_p; p*inv_p] per batch;
    # moving[b, q] = [A*x + B*q + c'; 4B] (unscaled)
    stat_all = np.stack(
        [inv[:, 2, :], p[None, :] * inv[:, 2, :]], axis=1
    )                                                  # [B, 2, 128]
    mov0 = (
        A[:, 2, None, None] * x[None, None, :]
        + Bc[:, 2, None, None] * q[None, :, None]
        + const[:, 2, None, None]
    )                                                  # [B, RPP, W]
    mov1 = np.broadcast_to(4.0 * Bc[:, 2, None, None], mov0.shape)

    in_maps, scls = [], []
    for c in range(N_CORES):
        g0 = c * BPC
        sl = slice(g0, g0 + BPC)
        bias_c = np.ascontiguousarray(
            bias_all[sl].reshape(BPC * 2 * RPP, 128).T.astype(np.float32)
        )                                              # [128, BPC*2*RPP]
        scale_c = np.ascontiguousarray(
            scale_all[sl].reshape(BPC * 2, 128).T.astype(np.float32)
        )                                              # [128, BPC*2]
        stat_c = np.ascontiguousarray(
            stat_all[sl].transpose(1, 0, 2).reshape(2, BPC * 128).astype(np.float16)
        )                                              # [2, BPC*128]
        mov_c = np.ascontiguousarray(
            np.stack(
                [mov0[sl].reshape(-1), mov1[sl].reshape(-1)], axis=0
            ).astype(np.float16)
        )                                              # [2, BPC*RPP*W]
        in_maps.append(
            {
                "depth": depth16[sl, 0],               # [BPC, H, W] fp16
                "scale": scale_c,
                "bias": bias_c,
                "stat": stat_c,
                "mov": mov_c,
            }
        )
        scls.append(np.ascontiguousarray(scl[sl]))     # [BPC, 3, 128]
    return in_maps, scls


def _expected_inputs(nc):
    import concourse.mybir as _mybir

    names = set()
    for alloc in nc.m.functions[0].allocations:
        if (
            isinstance(alloc, _mybir.MemoryLocationSet)
            and alloc.kind == "ExternalInput"
        ):
            names.add(alloc.memorylocations[0].name)
    return names


def _run(nc, in_maps, scls, trace=False):
    global _LAST_RESULTS
    want = _expected_inputs(nc)
    in_maps = [{k: v for k, v in m.items() if k in want} for m in in_maps]
    res = run_bass_kernel_spmd(
        nc, in_maps, core_ids=list(range(N_CORES)), trace=trace
    )
    _LAST_RESULTS = res
    out = np.empty((B, 4, HW), dtype=np.float32)
    for c in range(N_CORES):
        q = np.asarray(res.results[c]["out"])          # int8 [BPC, 3, HW]
        blk = q.reshape(BPC, 3, 128, CH).astype(np.float32)
        blk *= scls[c][:, :, :, None]                  # dequantize
        out[c * BPC : (c + 1) * BPC, :3] = blk.reshape(BPC, 3, HW)
    out[:, 3, :] = 1.0
    return out


def kernel(depth, inv_K, dxy):
    global _nc_cache
    in_maps, scls = _make_in_maps(depth, inv_K, dxy)
    if _nc_cache is None:
        _nc_cache = _build()
    return _run(_nc_cache, in_maps, scls, trace=_TRACE)


# revision 42
# speedup vs baseline: 1.2872x; 1.2872x over previous
"""Trainium2 Bass kernel for BackprojectDepth.

out[b, i, y*W+x] = depth[b, 0, y, x] * (K[b,i,0]*(x+dx[b]) + K[b,i,1]*(y+dy[b]) + K[b,i,2])   for i in 0..2
out[b, 3, :]    = 1.0

Sharding: pure data parallel over batch (32 batches -> 4 per core on 8 cores).

Memory-bound; the device program minimizes wire bytes (fp16 depth in, fp16
planes 0..2 out, constant ones-plane filled host-side during the gather =>
16 MB/core instead of 40 MB) and keeps the ~360 GB/s wire busy end-to-end:

  * layout: partition p holds image rows 4p..4p+3 (depth[b] / out[b,i] are
    the plain row-major reshape [128, 4096]); outs move as half-plane
    [128, 2048] DMAs (4 KB per-partition descriptors) so the out stream
    starts early and flows smoothly.
  * planes 0/1: lin = xg*A + bias on the scalar (ACT) engine per 1024-col
    chunk (int32 x-ramp input - measured faster than fp16 - fp16 out,
    1135 ns/op), then half-plane DVE multiplies by depth (~1.4 us each).
  * plane 2: lin on the tensor engine as K=2 matmuls (stationary [1; p]
    fixed, moving [A*x + B*q + c'; 4B] host-precomputed per (b,q)) into
    4-bank PSUM tiles, drained by [128,2048] DVE multiplies (~2.2 us).

Measured engine budgets/core: ACT 36.3 us, DVE ~39 us, PE ~22 us, wire
~45 us => DMA-bound when the pipeline stays dense.

Rings: sync = xg + depth[0] + plane 0/1 outs; scalar = consts + depth
prefetches (issued up-front, before any out can block them); gpsimd SWDGE
= plane 2 outs (gpsimd does no other work, avoiding SBUF contention).
"""

import numpy as np

import concourse.tile as tile
from concourse import bacc, mybir
from concourse.bass_utils import run_bass_kernel_spmd

N_CORES = 8
B, H, W = 32, 512, 1024
HW = H * W
BPC = B // N_CORES          # batches per core
RPP = H // 128              # image rows per partition (4)
CH = RPP * W                # cols per partition in plane layout (4096)
HC = CH // 2                # half-plane cols (2048)

F32 = mybir.dt.float32
F16 = mybir.dt.float16
I32 = mybir.dt.int32
I8 = mybir.dt.int8

_TRACE = False              # test.py may flip this for profiling
_LAST_RESULTS = None        # BassKernelResults from the last run (for test.py)

_nc_cache = None

DEFAULT_CFG = dict(
    dpool=4, opool=4, lpool=4, psum=2,
    plane_ring=("sync", "sync", "gpsimd"),
    o1h1_ring=None,          # optional override ring for plane-1 second half
)


def _build(**cfg_over):
    """Build + compile the per-core Bass program (SPMD: same NEFF, 8 cores)."""
    cfg = dict(DEFAULT_CFG, **cfg_over)
    nc = bacc.Bacc(
        "TRN2",
        target_bir_lowering=False,
        debug=False,
        enable_asserts=False,
        num_devices=N_CORES,
    )

    depth_d = nc.dram_tensor("depth", [BPC, H, W], F16, kind="ExternalInput")
    scale_d = nc.dram_tensor("scale", [128, BPC * 2], F32, kind="ExternalInput")
    bias_d = nc.dram_tensor("bias", [128, BPC * 2 * RPP], F32, kind="ExternalInput")
    stat_d = nc.dram_tensor("stat", [2, BPC * 128], F16, kind="ExternalInput")
    mov_d = nc.dram_tensor("mov", [2, BPC * RPP * W], F16, kind="ExternalInput")
    out16_d = nc.dram_tensor("out16", [BPC, 2, HW], F16, kind="ExternalOutput")
    out8_d = nc.dram_tensor("out8", [BPC, HW], I8, kind="ExternalOutput")

    rings = {"sync": nc.sync, "scalar": nc.scalar, "gpsimd": nc.gpsimd}

    with tile.TileContext(nc) as tc:
        with (
            tc.tile_pool(name="const", bufs=1) as cpool,
            tc.tile_pool(name="dpool", bufs=cfg["dpool"]) as dpool,
            tc.tile_pool(name="lpool", bufs=cfg["lpool"]) as lpool,
            tc.tile_pool(name="opool", bufs=cfg["opool"]) as opool,
            tc.psum_pool(name="ppool", bufs=cfg["psum"]) as ppool,
        ):
            # PE consts ride the gpsimd SWDGE ring: warms up its queue at
            # t~0 (else the first plane-2 out pays the ~7us cold start) and
            # keeps the scalar ring free for pure depth prefetch
            stat_t = cpool.tile([2, BPC * 128], F16)
            nc.gpsimd.dma_start(stat_t[:], stat_d.ap())
            mov_t = cpool.tile([2, BPC * RPP * W], F16)
            nc.gpsimd.dma_start(mov_t[:], mov_d.ap())
            # x-ramp on gpsimd (no DMA dependency); ACT converts i32 on read
            xg_t = cpool.tile([128, W], I32)
            nc.gpsimd.iota(xg_t[:], pattern=[[1, W]], base=0, channel_multiplier=0)
            sc_t = cpool.tile([128, BPC * 2], F32)
            nc.sync.dma_start(sc_t[:], scale_d.ap())
            bi_t = cpool.tile([128, BPC * 2 * RPP], F32)
            nc.sync.dma_start(bi_t[:], bias_d.ap())

            # partition p <-> image rows 4p..4p+3 (plain row-major reshape)
            depth_ap = depth_d.ap().rearrange("b (p q) w -> b p (q w)", p=128)
            out_ap = out16_d.ap().rearrange("b i (p j) -> b i p j", p=128)
            out8_ap = out8_d.ap().rearrange("b (p j) -> b p j", p=128)

            # all depth loads issued up-front so prefetch never queues
            # behind an out-DMA on the same ring; depth[0] lands in column
            # quarters split over both HWDGE rings so the first chunk
            # arrives ~2us sooner
            d_ts = []
            for b in range(BPC):
                d_t = dpool.tile([128, CH], F16)
                if b == 0:
                    for qtr, deng in enumerate((nc.sync, nc.scalar, nc.sync, nc.scalar)):
                        sl = slice(qtr * W, (qtr + 1) * W)
                        deng.dma_start(d_t[:, sl], depth_ap[b, :, sl])
                else:
                    nc.scalar.dma_start(d_t[:], depth_ap[b])
                d_ts.append(d_t)

            def act_lin(b, i):
                l_t = lpool.tile([128, CH], F16)
                col = 2 * b + i
                for q in range(RPP):
                    nc.scalar.activation(
                        l_t[:, q * W : (q + 1) * W],
                        xg_t[:],
                        mybir.ActivationFunctionType.Identity,
                        bias=bi_t[:, col * RPP + q : col * RPP + q + 1],
                        scale=sc_t[:, col : col + 1],
                    )
                return l_t

            def mul_and_store(b, i, h, o_t, lin_ap, d_t):
                sl = slice(h * HC, (h + 1) * HC)
                nc.vector.tensor_mul(o_t[:, sl], lin_ap[:, sl], d_t[:, sl])
                ring = rings[cfg["plane_ring"][i]]
                if i == 1 and h == 1 and cfg["o1h1_ring"]:
                    ring = rings[cfg["o1h1_ring"]]
                ring.dma_start(out_ap[b, i, :, sl], o_t[:, sl])

            def pe_plane(b, d_t):
                o2 = opool.tile([128, CH], I8)
                for hf in range(2):
                    ps = ppool.tile([128, HC], F32)
                    for s in range(4):
                        c0 = hf * HC + s * 512
                        q, xo = c0 // W, c0 % W
                        nc.tensor.matmul(
                            ps[:, s * 512 : (s + 1) * 512],
                            stat_t[:, b * 128 : (b + 1) * 128],
                            mov_t[:, (b * RPP + q) * W + xo : (b * RPP + q) * W + xo + 512],
                            start=True,
                            stop=True,
                        )
                    sl = slice(hf * HC, (hf + 1) * HC)
                    nc.vector.tensor_mul(o2[:, sl], ps[:], d_t[:, sl])
                    rings[cfg["plane_ring"][2]].dma_start(out8_ap[b, :, sl], o2[:, sl])

            def act_plane(b, i, d_t, quarters=False):
                l_t = act_lin(b, i)
                o_t = opool.tile([128, CH], F16)
                if quarters:
                    # batch 0 plane 0: quarter-granularity so the first out
                    # bytes hit the wire as soon as the first depth quarter
                    # and lin chunk exist
                    for qtr in range(RPP):
                        sl = slice(qtr * W, (qtr + 1) * W)
                        nc.vector.tensor_mul(o_t[:, sl], l_t[:, sl], d_t[:, sl])
                        rings[cfg["plane_ring"][i]].dma_start(
                            out_ap[b, i, :, sl], o_t[:, sl]
                        )
                    return
                for h in range(2):
                    mul_and_store(b, i, h, o_t, l_t[:], d_t)

            for b in range(BPC):
                d_t = d_ts[b]
                act_plane(b, 0, d_t, quarters=(b == 0))
                pe_plane(b, d_t)
                act_plane(b, 1, d_t)

    nc.compile()
    return nc


def _make_in_maps(depth, inv_K, dxy):
    depth16 = np.ascontiguousarray(
        np.asarray(depth, dtype=np.float32).astype(np.float16)
    )
    K = np.asarray(inv_K, dtype=np.float64)
    dx = np.asarray(dxy, dtype=np.float64)

    # Per-batch affine coefficients: cam_i = A*x' + B*y' + C with x'=x+dx, y'=y+dy
    A = K[:, :3, 0]                                   # [B, 3]
    Bc = K[:, :3, 1]
    C = K[:, :3, 2]
    const = A * dx[:, None, 0] + Bc * dx[:, None, 1] + C   # [B, 3]

    p = np.arange(128, dtype=np.float64)
    q = np.arange(RPP, dtype=np.float64)
    x = np.arange(W, dtype=np.float64)

    # int8 output quantization: per (b, i, 4-row-group p) bound
    # s[b,i,p] = max |lin| over the group (lin affine in x,y => corners),
    # fold inv = 126/s into the lin tables so the device's existing
    # multiply emits pre-scaled int8; host dequantizes by s/126.
    xv = np.array([0.0, W - 1.0])
    yc = 4.0 * p[:, None] + np.array([0.0, 3.0])[None, :]          # [128, 2]
    lin_c = (
        A[:, :, None, None, None] * xv[None, None, None, None, :]
        + Bc[:, :, None, None, None] * yc[None, None, :, :, None]
        + const[:, :, None, None, None]
    )                                                  # [B, 3, 128, 2, 2]
    s_all = np.abs(lin_c).max(axis=(3, 4))             # [B, 3, 128]
    inv = np.minimum(126.0 / np.maximum(s_all, 1e-9), 500.0)
    scl = (1.0 / inv).astype(np.float32)               # host dequant factors

    # ACT path (planes 0/1, fp16 out -- int8 would halve DVE throughput
    # for SBUF-sourced muls): unscaled lin tables
    bias_all = (
        Bc[:, :2, None, None] * (4.0 * p[None, None, None, :] + q[None, None, :, None])
        + const[:, :2, None, None]
    )                                                  # [B, 2, RPP, 128]
    scale_all = np.broadcast_to(A[:, :2, None], (B, 2, 128))
    # PE path (plane 2): stationary rows [inv_p; p*inv_p] per batch;
    # moving[b, q] = [A*x + B*q + c'; 4B] (unscaled)
    stat_all = np.stack(
        [inv[:, 2, :], p[None, :] * inv[:, 2, :]], axis=1
    )                                                  # [B, 2, 128]
    mov0 = (
        A[:, 2, None, None] * x[None, None, :]
        + Bc[:, 2, None, None] * q[None, :, None]
        + const[:, 2, None, None]
    )                                                  # [B, RPP, W]
    mov1 = np.broadcast_to(4.0 * Bc[:, 2, None, None], mov0.shape)

    in_maps, scls = [], []
    for c in range(N_CORES):
        g0 = c * BPC
        sl = slice(g0, g0 + BPC)
        bias_c = np.ascontiguousarray(
            bias_all[sl].reshape(BPC * 2 * RPP, 128).T.astype(np.float32)
        )                                              # [128, BPC*2*RPP]
        scale_c = np.ascontiguousarray(
            scale_all[sl].reshape(BPC * 2, 128).T.astype(np.float32)
        )                                              # [128, BPC*2]
        stat_c = np.ascontiguousarray(
            stat_all[sl].transpose(1, 0, 2).reshape(2, BPC * 128).astype(np.float16)
        )                                              # [2, BPC*128]
        mov_c = np.ascontiguousarray(
            np.stack(
                [mov0[sl].reshape(-1), mov1[sl].reshape(-1)], axis=0
            ).astype(np.float16)
        )                                              # [2, BPC*RPP*W]
        in_maps.append(
            {
                "depth": depth16[sl, 0],               # [BPC, H, W] fp16
                "scale": scale_c,
                "bias": bias_c,
                "stat": stat_c,
                "mov": mov_c,
            }
        )
        scls.append(np.ascontiguousarray(scl[sl, 2]))  # [BPC, 128] (plane 2)
    return in_maps, scls


def _expected_inputs(nc):
    import concourse.mybir as _mybir

    names = set()
    for alloc in nc.m.functions[0].allocations:
        if (
            isinstance(alloc, _mybir.MemoryLocationSet)
            and alloc.kind == "ExternalInput"
        ):
            names.add(alloc.memorylocations[0].name)
    return names


def _run(nc, in_maps, scls, trace=False):
    global _LAST_RESULTS
    want = _expected_inputs(nc)
    in_maps = [{k: v for k, v in m.items() if k in want} for m in in_maps]
    res = run_bass_kernel_spmd(
        nc, in_maps, core_ids=list(range(N_CORES)), trace=trace
    )
    _LAST_RESULTS = res
    out = np.empty((B, 4, HW), dtype=np.float32)
    for c in range(N_CORES):
        bs = slice(c * BPC, (c + 1) * BPC)
        out[bs, :2] = res.results[c]["out16"]          # fp16 -> f32
        q = np.asarray(res.results[c]["out8"])         # int8 [BPC, HW]
        blk = q.reshape(BPC, 128, CH).astype(np.float32)
        blk *= scls[c][:, :, None]                     # dequantize plane 2
        out[bs, 2] = blk.reshape(BPC, HW)
    out[:, 3, :] = 1.0
    return out


def kernel(depth, inv_K, dxy):
    global _nc_cache
    in_maps, scls = _make_in_maps(depth, inv_K, dxy)
    if _nc_cache is None:
        _nc_cache = _build()
    return _run(_nc_cache, in_maps, scls, trace=_TRACE)
